# revision 1
# baseline (speedup 1.0000x reference)
"""GAT block (graph attention) Bass/Tile kernel for Trainium2, 8 NeuronCores.

Full-input contract: kernel(x=(8,2048,128), W=(128,64), a=(128,1)) -> (8,2048,64).
Sharding: data-parallel over batch — one batch element per core, W/a replicated.

Per-core math (N=2048, Fin=128, Fout=64):
  h  = x @ W                               (N, Fout)
  s1 = h @ a[:64, 0],  s2 = h @ a[64:, 0]  (N,)
  e[i, j]   = leakyrelu(s1[i] + s2[j], 0.2)
  att       = softmax(e, axis=0)  (normalize over i for each column j)
  out       = leakyrelu(att @ h, 0.2)

Implementation notes:
  * attention matrix kept transposed: Pt[j, i] = exp(lrelu(s1[i] + s2[j])).
    Per 128-row tile it's exactly TWO ScalarE (ACT) ops:
      L = Prelu(S1_bcast + s2[j])   (bias = per-partition s2 column, alpha=0.2,
                                     parametric_relu lives in the exp table set
                                     so no activation-table reloads)
      P = Exp(L) with accum_out     (fused free-dim reduction = softmax denom)
    No max-subtraction: |s1+s2| <~ 15 so exp is far from fp32 overflow; this
    matches jax softmax to fp32 rounding.
  * out[i] = sum_j Pt[j, i] * (h[j]/denom[j]) via PE accumulation in PSUM,
    P and scaled-h cast to bf16 for the big matmul (errors average out over
    the j-sum; measured end-to-end rel err ~1e-3).
"""

import numpy as np
from contextlib import ExitStack

import concourse.bass as bass
import concourse.mybir as mybir
import concourse.tile as tile
from concourse import bacc
from concourse._compat import with_exitstack
from concourse.bass_utils import run_bass_kernel_spmd
from concourse.masks import make_identity

F32 = mybir.dt.float32
BF16 = mybir.dt.bfloat16
AF = mybir.ActivationFunctionType
ALU = mybir.AluOpType

N = 2048
FIN = 128
FOUT = 64
P = 128
T = N // P  # 16 row tiles
NEG_SLOPE = 0.2
N_CORES = 8

# ACT lrelu-tile assignment: tiles whose Prelu runs on DVE instead of ACT
# (rebalance: ACT exp pass is the serial bottleneck). Empty in v1.
DVE_LRELU_TILES: set = set()


@with_exitstack
def _gat_body(ctx: ExitStack, tc: tile.TileContext, x, w, a, out):
    nc = tc.nc

    const = ctx.enter_context(tc.tile_pool(name="const", bufs=1))
    xin = ctx.enter_context(tc.tile_pool(name="xin", bufs=4))
    lpool = ctx.enter_context(tc.tile_pool(name="lrelu", bufs=3))
    opool = ctx.enter_context(tc.tile_pool(name="outs", bufs=3))
    ps_setup = ctx.enter_context(tc.tile_pool(name="ps_setup", bufs=3, space="PSUM"))
    ps_main = ctx.enter_context(tc.tile_pool(name="ps_main", bufs=3, space="PSUM"))

    # ---- constants / persistent tiles ----
    ident = const.tile([P, P], F32)
    make_identity(nc, ident)
    w_sb = const.tile([FIN, FOUT], F32)
    nc.sync.dma_start(w_sb[:], w)
    acol = const.tile([FOUT, 2], F32)  # [:,0]=a1, [:,1]=a2
    nc.sync.dma_start(acol[:, 0:1], a[0:FOUT, :])
    nc.sync.dma_start(acol[:, 1:2], a[FOUT:, :])
    ones_row = const.tile([1, P], F32)
    nc.vector.memset(ones_row[:], 1.0)

    xT = const.tile([P, T, P], F32)        # x transposed: [k, t, n]
    h_all = const.tile([P, T, FOUT], F32)  # h natural: [n(part), t, f]
    hs_bf = const.tile([P, T, FOUT], BF16)  # h/denom in bf16
    hT = const.tile([FOUT, N], F32)        # h transposed: [f, n]
    s1b = const.tile([P, N], F32)          # s1 broadcast along partitions
    s2col = const.tile([P, T], F32)        # s2 in column layout per tile
    srow = const.tile([1, N], F32)         # s1 as a row
    denom = const.tile([P, T], F32)
    rden = const.tile([P, T], F32)
    p_all = const.tile([P, T, N], BF16)    # attention numerator, transposed

    # ---- stage A/B: load x, transpose, h = x@W (both layouts), scores ----
    for t in range(T):
        xn = xin.tile([P, FIN], F32, tag="xn")
        nc.sync.dma_start(xn[:], x[t * P:(t + 1) * P, :])
        psT = ps_setup.tile([P, P], F32, tag="ps")
        nc.tensor.transpose(psT[:], xn[:], ident[:])
        nc.vector.tensor_copy(xT[:, t, :], psT[:])

        ps_h = ps_setup.tile([P, FOUT], F32, tag="ps")
        nc.tensor.matmul(ps_h[:], lhsT=xT[:, t, :], rhs=w_sb[:], start=True, stop=True)
        nc.vector.tensor_copy(h_all[:, t, :], ps_h[:])

        ps_hT = ps_setup.tile([FOUT, P], F32, tag="ps")
        nc.tensor.matmul(ps_hT[:], lhsT=w_sb[:], rhs=xT[:, t, :], start=True, stop=True)
        nc.vector.tensor_copy(hT[:, t * P:(t + 1) * P], ps_hT[:])

        # s2 column for this tile: (128, 1) = hT_tile.T @ a2
        ps_s = ps_setup.tile([P, 1], F32, tag="ps")
        nc.tensor.matmul(ps_s[:], lhsT=hT[:, t * P:(t + 1) * P], rhs=acol[:, 1:2],
                         start=True, stop=True)
        nc.vector.tensor_copy(s2col[:, t:t + 1], ps_s[:])

    # s1 as a row, then broadcast to all 128 partitions via K=1 matmul
    for c in range(N // 512):
        ps_r = ps_setup.tile([1, 512], F32, tag="ps")
        nc.tensor.matmul(ps_r[:], lhsT=acol[:, 0:1], rhs=hT[:, c * 512:(c + 1) * 512],
                         start=True, stop=True)
        nc.vector.tensor_copy(srow[:, c * 512:(c + 1) * 512], ps_r[:])
        ps_b = ps_setup.tile([P, 512], F32, tag="ps")
        nc.tensor.matmul(ps_b[:], lhsT=ones_row[:], rhs=srow[:, c * 512:(c + 1) * 512],
                         start=True, stop=True)
        nc.vector.tensor_copy(s1b[:, c * 512:(c + 1) * 512], ps_b[:])

    # ---- stage C: Pt tiles + denominators ----
    for t in range(T):
        if t in DVE_LRELU_TILES:
            # z = s1b + s2[j] on DVE (2x tensor_scalar), lrelu via STT
            z02 = lpool.tile([P, N], F32, tag="z02")
            nc.vector.tensor_scalar(z02[:], s1b[:], s2col[:, t:t + 1], NEG_SLOPE,
                                    op0=ALU.add, op1=ALU.mult)
            l_t = lpool.tile([P, N], F32, tag="l")
            nc.vector.scalar_tensor_tensor(l_t[:], in0=s1b[:], scalar=s2col[:, t:t + 1],
                                           in1=z02[:], op0=ALU.add, op1=ALU.max)
        else:
            l_t = lpool.tile([P, N], F32, tag="l")
            nc.scalar.activation(l_t[:], s1b[:], AF.Prelu,
                                 bias=s2col[:, t:t + 1], scale=1.0, alpha=NEG_SLOPE)
        nc.scalar.activation(p_all[:, t, :], l_t[:], AF.Exp,
                             accum_out=denom[:, t:t + 1])

    # ---- stage D: 1/denom, scale h ----
    nc.vector.reciprocal(rden[:], denom[:])
    for t in range(T):
        nc.vector.tensor_scalar_mul(hs_bf[:, t, :], h_all[:, t, :], rden[:, t:t + 1])

    # ---- stage E: out[i-tile] = sum_t Pt[t][:, i-tile].T @ hs[t], lrelu, store ----
    for ti in range(T):
        ps_o = ps_main.tile([P, FOUT], F32, tag="ps_o")
        for t in range(T):
            nc.tensor.matmul(ps_o[:], lhsT=p_all[:, t, ti * P:(ti + 1) * P],
                             rhs=hs_bf[:, t, :], start=(t == 0), stop=(t == T - 1))
        o_c = opool.tile([P, FOUT], F32, tag="o_c")
        nc.vector.tensor_copy(o_c[:], ps_o[:])
        o_f = opool.tile([P, FOUT], F32, tag="o_f")
        nc.vector.scalar_tensor_tensor(o_f[:], in0=o_c[:], scalar=NEG_SLOPE,
                                       in1=o_c[:], op0=ALU.mult, op1=ALU.max)
        nc.sync.dma_start(out[ti * P:(ti + 1) * P, :], o_f[:])


_NC_CACHE = {}


def _build_nc():
    if "nc" in _NC_CACHE:
        return _NC_CACHE["nc"]
    nc = bacc.Bacc("TRN2", target_bir_lowering=False, debug=False)
    x = nc.dram_tensor("x", (N, FIN), F32, kind="ExternalInput").ap()
    w = nc.dram_tensor("w", (FIN, FOUT), F32, kind="ExternalInput").ap()
    a = nc.dram_tensor("a", (2 * FOUT, 1), F32, kind="ExternalInput").ap()
    out = nc.dram_tensor("out", (N, FOUT), F32, kind="ExternalOutput").ap()
    with tile.TileContext(nc) as tc:
        _gat_body(tc, x, w, a, out)
    nc.compile()
    _NC_CACHE["nc"] = nc
    return nc


def kernel(x, W, a):
    x = np.ascontiguousarray(np.asarray(x), dtype=np.float32)
    W = np.ascontiguousarray(np.asarray(W), dtype=np.float32)
    a = np.ascontiguousarray(np.asarray(a), dtype=np.float32)
    assert x.shape == (N_CORES, N, FIN), x.shape
    nc = _build_nc()
    in_maps = [{"x": x[c], "w": W, "a": a} for c in range(N_CORES)]
    res = run_bass_kernel_spmd(nc, in_maps, core_ids=list(range(N_CORES)))
    return np.stack([res.results[c]["out"] for c in range(N_CORES)], axis=0)


# revision 10
# speedup vs baseline: 1.7370x; 1.7370x over previous
"""GAT block (graph attention) Bass/Tile kernel for Trainium2, 8 NeuronCores.

Full-input contract: kernel(x=(8,2048,128), W=(128,64), a=(128,1)) -> (8,2048,64).
Sharding: data-parallel over batch — one batch element per core, W/a replicated.

Per-core math (N=2048, Fin=128, Fout=64):
  h  = x @ W                               (N, Fout)
  s1 = h @ a[:64, 0],  s2 = h @ a[64:, 0]  (N,)
  e[i, j]   = leakyrelu(s1[i] + s2[j], 0.2)
  att       = softmax(e, axis=0)  (normalize over i for each column j)
  out       = leakyrelu(att @ h, 0.2)

Implementation notes:
  * attention matrix kept transposed: Pt[j, i] = exp(lrelu(s1[i] + s2[j])).
    leakyrelu tiles are split across ACT/DVE/GPSIMD to balance engines; the
    exp runs on ACT (Prelu/parametric_relu shares the exp activation-table
    set so there are no table reloads) with accum_out giving the softmax
    denominator for free.
  * No max-subtraction: |s1+s2| <~ 15 so exp is far from fp32 overflow; this
    matches jax softmax to fp32 rounding.
  * setup matmuls run in float32r (single-pass TF32-ish) — fp32 matmuls on
    TRN2 are two-pass (LOW/HIGH) and twice the cost.
  * out is accumulated transposed (hpT[f, i] in 4 PSUM banks, one per
    512-wide i-chunk) so the 64 bf16 matmuls overlap the ACT/DVE stream
    tile-by-tile; the host un-transposes the (64, 2048) result.
"""

import numpy as np
from contextlib import ExitStack

import concourse.bass as bass
import concourse.mybir as mybir
import concourse.tile as tile
from concourse import bacc
from concourse._compat import with_exitstack
from concourse.bass_utils import run_bass_kernel_spmd
from concourse.masks import make_identity

F32 = mybir.dt.float32
F32R = mybir.dt.float32r
BF16 = mybir.dt.bfloat16
AF = mybir.ActivationFunctionType
ALU = mybir.AluOpType

N = 2048
FIN = 128
FOUT = 64
P = 128
T = N // P          # 16 row tiles
NC = N // 512       # 4 i-chunks for the output accumulation
NEG_SLOPE = 0.2
N_CORES = 8

# leakyrelu-tile engine assignment (exp always runs on ACT)
ACT_TILES = {0, 1, 2, 3, 4}
GPS_TILES = set()


@with_exitstack
def _gat_body(ctx: ExitStack, tc: tile.TileContext, x, w, a, out):
    nc = tc.nc

    const = ctx.enter_context(tc.tile_pool(name="const", bufs=1))
    xin = ctx.enter_context(tc.tile_pool(name="xin", bufs=8))
    lpool = ctx.enter_context(tc.tile_pool(name="lrelu", bufs=4))
    dpool = ctx.enter_context(tc.tile_pool(name="denoms", bufs=2 * T))
    ps_tr = ctx.enter_context(tc.tile_pool(name="ps_tr", bufs=2, space="PSUM"))
    ps_mm = ctx.enter_context(tc.tile_pool(name="ps_mm", bufs=2, space="PSUM"))
    ps_out = ctx.enter_context(tc.tile_pool(name="ps_out", bufs=1, space="PSUM"))

    # ---- constants / persistent tiles ----
    ident = const.tile([P, P], F32)
    make_identity(nc, ident)
    w_raw = const.tile([FIN, FOUT], F32)
    nc.sync.dma_start(w_raw[:], w)
    w_sb = const.tile([FIN, FOUT], F32R)
    nc.vector.tensor_copy(w_sb[:], w_raw[:])
    a_raw = const.tile([FOUT, 2], F32)  # [:,0]=a1, [:,1]=a2
    nc.sync.dma_start(a_raw[:, 0:1], a[0:FOUT, :])
    nc.sync.dma_start(a_raw[:, 1:2], a[FOUT:, :])
    acol = const.tile([FOUT, 2], F32R)
    nc.vector.tensor_copy(acol[:], a_raw[:])
    ones_raw = const.tile([1, P], F32)
    nc.vector.memset(ones_raw[:], 1.0)
    ones_row = const.tile([1, P], F32R)
    nc.vector.tensor_copy(ones_row[:], ones_raw[:])

    xT = const.tile([P, T, P], F32R)        # x transposed: [k, t, n]
    h_all = const.tile([P, T, FOUT], F32)   # h natural: [n(part), t, f]
    hs_bf = const.tile([P, T, FOUT], BF16)  # h/denom in bf16
    wa = const.tile([FIN, 2], F32R)         # W @ [a1, a2]: scores = x @ wa
    s1b = const.tile([P, N], F32)           # s1 broadcast along partitions
    s12 = const.tile([P, T, 2], F32)        # [s1, s2] column layout per tile
    srow = const.tile([2, N], F32R)         # [s1, s2] as rows
    p_all = const.tile([P, T, N], BF16)     # attention numerator, transposed
    o_sb = const.tile([FOUT, N], F32)       # output transposed

    # wa = W @ [a1, a2]  (via wT = W.T then wT.T-contraction over f)
    ps_wT = ps_mm.tile([FOUT, FIN], F32, tag="ps_m")
    nc.tensor.transpose(ps_wT[:], w_raw[:], ident[:])
    wT = const.tile([FOUT, FIN], F32R)
    nc.vector.tensor_copy(wT[:], ps_wT[:])
    ps_wa = ps_mm.tile([FIN, 2], F32, tag="ps_m")
    nc.tensor.matmul(ps_wa[:], lhsT=wT[:], rhs=acol[:], start=True, stop=True)
    nc.vector.tensor_copy(wa[:], ps_wa[:])

    # ---- setup: load x, transpose, h, scores (cols and rows) ----
    for t in range(T):
        xn = xin.tile([P, FIN], F32, tag="xn")
        nc.sync.dma_start(xn[:], x[t * P:(t + 1) * P, :])
        psT = ps_tr.tile([P, P], F32, tag="ps_t")
        nc.tensor.transpose(psT[:], xn[:], ident[:])
        nc.vector.tensor_copy(xT[:, t, :], psT[:])

    for t in range(T):
        ps_h = ps_mm.tile([P, FOUT], F32, tag="ps_m")
        nc.tensor.matmul(ps_h[:], lhsT=xT[:, t, :], rhs=w_sb[:],
                         start=True, stop=True)
        nc.vector.tensor_copy(h_all[:, t, :], ps_h[:])

        # score columns for this tile: (128, 2) = x_t @ wa
        ps_s = ps_mm.tile([P, 2], F32, tag="ps_m")
        nc.tensor.matmul(ps_s[:], lhsT=xT[:, t, :], rhs=wa[:],
                         start=True, stop=True)
        nc.vector.tensor_copy(s12[:, t, :], ps_s[:])

        # score rows for this tile: (2, 128) = wa.T @ x_t.T
        ps_r = ps_mm.tile([2, P], F32, tag="ps_m")
        nc.tensor.matmul(ps_r[:], lhsT=wa[:], rhs=xT[:, t, :],
                         start=True, stop=True)
        nc.vector.tensor_copy(srow[:, t * P:(t + 1) * P], ps_r[:])

    # broadcast s1 row to all 128 partitions via K=1 matmul
    for c in range(NC):
        sl = slice(c * 512, (c + 1) * 512)
        ps_b = ps_mm.tile([P, 512], F32, tag="ps_m")
        nc.tensor.matmul(ps_b[:], lhsT=ones_row[:], rhs=srow[0:1, sl],
                         start=True, stop=True)
        nc.vector.tensor_copy(s1b[:, sl], ps_b[:])

    # ---- main: per j-tile lrelu -> exp(+denom) -> scale h -> accumulate out ----
    hp_ps = [ps_out.tile([FOUT, 512], F32, tag=f"hp{c}", name=f"hp{c}")
             for c in range(NC)]

    for t in range(T):
        if t in ACT_TILES:
            l_t = lpool.tile([P, N], F32, tag="l")
            nc.scalar.activation(l_t[:], s1b[:], AF.Prelu,
                                 bias=s12[:, t, 1:2], scale=1.0, alpha=NEG_SLOPE)
        elif t in GPS_TILES:
            # Pool engine has no scalar_tensor_tensor; use TS + TS + TT
            z_t = lpool.tile([P, N], F32, tag="z")
            nc.gpsimd.tensor_scalar(z_t[:], s1b[:], s12[:, t, 1:2], None,
                                    op0=ALU.add)
            z02 = lpool.tile([P, N], F32, tag="z02")
            nc.gpsimd.tensor_scalar(z02[:], z_t[:], NEG_SLOPE, None, op0=ALU.mult)
            l_t = lpool.tile([P, N], F32, tag="l")
            nc.gpsimd.tensor_tensor(l_t[:], z_t[:], z02[:], ALU.max)
        else:
            z02 = lpool.tile([P, N], F32, tag="z02")
            nc.vector.tensor_scalar(z02[:], s1b[:], s12[:, t, 1:2], NEG_SLOPE,
                                    op0=ALU.add, op1=ALU.mult)
            l_t = lpool.tile([P, N], F32, tag="l")
            nc.vector.scalar_tensor_tensor(l_t[:], in0=s1b[:],
                                           scalar=s12[:, t, 1:2],
                                           in1=z02[:], op0=ALU.add, op1=ALU.max)

        den_t = dpool.tile([P, 1], F32, tag="den")
        nc.scalar.activation(p_all[:, t, :], l_t[:], AF.Exp, accum_out=den_t[:])

        rden_t = dpool.tile([P, 1], F32, tag="rden")
        nc.vector.reciprocal(rden_t[:], den_t[:])
        nc.vector.tensor_scalar_mul(hs_bf[:, t, :], h_all[:, t, :], rden_t[:])

        for c in range(NC):
            nc.tensor.matmul(hp_ps[c][:], lhsT=hs_bf[:, t, :],
                             rhs=p_all[:, t, c * 512:(c + 1) * 512],
                             start=(t == 0), stop=(t == T - 1))

    # ---- epilogue: leakyrelu on ACT straight from PSUM, DMA out transposed ----
    for c in range(NC):
        sl = slice(c * 512, (c + 1) * 512)
        nc.scalar.activation(o_sb[:, sl], hp_ps[c][:], AF.Prelu,
                             bias=0.0, scale=1.0, alpha=NEG_SLOPE)
        nc.sync.dma_start(out[:, sl], o_sb[:, sl])


_NC_CACHE = {}


def _build_nc():
    if "nc" in _NC_CACHE:
        return _NC_CACHE["nc"]
    nc = bacc.Bacc("TRN2", target_bir_lowering=False, debug=False)
    x = nc.dram_tensor("x", (N, FIN), F32, kind="ExternalInput").ap()
    w = nc.dram_tensor("w", (FIN, FOUT), F32, kind="ExternalInput").ap()
    a = nc.dram_tensor("a", (2 * FOUT, 1), F32, kind="ExternalInput").ap()
    # transposed output; the host un-transposes
    out = nc.dram_tensor("out", (FOUT, N), F32, kind="ExternalOutput").ap()
    with tile.TileContext(nc) as tc:
        _gat_body(tc, x, w, a, out)
    nc.compile()
    _NC_CACHE["nc"] = nc
    return nc


def kernel(x, W, a):
    x = np.ascontiguousarray(np.asarray(x), dtype=np.float32)
    W = np.ascontiguousarray(np.asarray(W), dtype=np.float32)
    a = np.ascontiguousarray(np.asarray(a), dtype=np.float32)
    assert x.shape == (N_CORES, N, FIN), x.shape
    nc = _build_nc()
    in_maps = [{"x": x[c], "w": W, "a": a} for c in range(N_CORES)]
    res = run_bass_kernel_spmd(nc, in_maps, core_ids=list(range(N_CORES)))
    return np.stack([res.results[c]["out"].T.copy() for c in range(N_CORES)], axis=0)


# revision 11
# speedup vs baseline: 1.8425x; 1.0607x over previous
"""GAT block (graph attention) Bass/Tile kernel for Trainium2, 8 NeuronCores.

Full-input contract: kernel(x=(8,2048,128), W=(128,64), a=(128,1)) -> (8,2048,64).
Sharding: data-parallel over batch — one batch element per core, W/a replicated.

Per-core math (N=2048, Fin=128, Fout=64):
  h  = x @ W                               (N, Fout)
  s1 = h @ a[:64, 0],  s2 = h @ a[64:, 0]  (N,)
  e[i, j]   = leakyrelu(s1[i] + s2[j], 0.2)
  att       = softmax(e, axis=0)  (normalize over i for each column j)
  out       = leakyrelu(att @ h, 0.2)

Implementation notes:
  * attention matrix kept transposed: Pt[j, i] = exp(lrelu(s1[i] + s2[j])).
    leakyrelu tiles are split across ACT and DVE(+GPSIMD) to balance engines;
    the exp runs on ACT (Prelu/parametric_relu shares the exp activation-table
    set so there are no table reloads) with accum_out giving the softmax
    denominator for free.
  * No max-subtraction: |s1+s2| <~ 15 so exp is far from fp32 overflow; this
    matches jax softmax to fp32 rounding.
  * setup matmuls run in float32r (single-pass) — fp32 matmuls on TRN2 are
    two-pass (LOW/HIGH) and twice the cost.  h and both score projections
    come from one stationary [W | W@a1 | W@a2] per x-tile.
  * out is accumulated transposed (hpT[f, i] in 4 PSUM banks, one per
    512-wide i-chunk) so the 64 bf16 matmuls overlap the ACT/DVE stream
    tile-by-tile; the host un-transposes the (64, 2048) result.
"""

import numpy as np
from contextlib import ExitStack

import concourse.bass as bass
import concourse.mybir as mybir
import concourse.tile as tile
from concourse import bacc
from concourse._compat import with_exitstack
from concourse.bass_utils import run_bass_kernel_spmd
from concourse.masks import make_identity

F32 = mybir.dt.float32
F32R = mybir.dt.float32r
BF16 = mybir.dt.bfloat16
AF = mybir.ActivationFunctionType
ALU = mybir.AluOpType

N = 2048
FIN = 128
FOUT = 64
P = 128
T = N // P          # 16 row tiles
NC = N // 512       # 4 i-chunks for the output accumulation
NEG_SLOPE = 0.2
N_CORES = 8

# leakyrelu-tile engine assignment (exp always runs on ACT; non-ACT tiles
# compute z*0.2 on GPSIMD and the max on DVE)
ACT_TILES = {0, 1, 2, 3, 4}


@with_exitstack
def _gat_body(ctx: ExitStack, tc: tile.TileContext, x, w, a, out):
    nc = tc.nc

    const = ctx.enter_context(tc.tile_pool(name="const", bufs=1))
    xin = ctx.enter_context(tc.tile_pool(name="xin", bufs=8))
    lpool = ctx.enter_context(tc.tile_pool(name="lrelu", bufs=4))
    dpool = ctx.enter_context(tc.tile_pool(name="denoms", bufs=2 * T))

    # ---- constants / persistent tiles ----
    ident = const.tile([P, P], F32)
    make_identity(nc, ident)
    w_raw = const.tile([FIN, FOUT], F32)
    nc.sync.dma_start(w_raw[:], w)
    a_raw = const.tile([FOUT, 2], F32)  # [:,0]=a1, [:,1]=a2
    nc.sync.dma_start(a_raw[:, 0:1], a[0:FOUT, :])
    nc.sync.dma_start(a_raw[:, 1:2], a[FOUT:, :])
    acol = const.tile([FOUT, 2], F32R)
    nc.vector.tensor_copy(acol[:], a_raw[:])
    ones_raw = const.tile([1, P], F32)
    nc.vector.memset(ones_raw[:], 1.0)
    ones_row = const.tile([1, P], F32R)
    nc.vector.tensor_copy(ones_row[:], ones_raw[:])

    xT = const.tile([P, T, P], F32R)        # x transposed: [k, t, n]
    hs12 = const.tile([P, T, FOUT + 2], F32)  # [h | s1 s2 cols] per tile
    hs_bf = const.tile([P, T, FOUT], BF16)  # h/denom in bf16
    wsa = const.tile([FIN, FOUT + 2], F32R)  # [W | W@a1 | W@a2]
    s1b = const.tile([P, N], F32)           # s1 broadcast along partitions
    srow = const.tile([2, N], F32R)         # [s1, s2] as rows
    p_all = const.tile([P, T, N], BF16)     # attention numerator, transposed
    o_sb = const.tile([FOUT, N], F32)       # output transposed

    with tc.tile_pool(name="ps_tr", bufs=3, space="PSUM") as ps_tr, \
         tc.tile_pool(name="ps_mm", bufs=2, space="PSUM") as ps_mm:
        # wsa = [W | W @ [a1, a2]]  (wa via wT = W.T, contraction over f)
        nc.vector.tensor_copy(wsa[:, 0:FOUT], w_raw[:])
        ps_wT = ps_mm.tile([FOUT, FIN], F32, tag="ps_h")
        nc.tensor.transpose(ps_wT[:], w_raw[:], ident[:])
        wT = const.tile([FOUT, FIN], F32R)
        nc.vector.tensor_copy(wT[:], ps_wT[:])
        ps_wa = ps_mm.tile([FIN, 2], F32, tag="ps_r")
        nc.tensor.matmul(ps_wa[:], lhsT=wT[:], rhs=acol[:], start=True, stop=True)
        nc.vector.tensor_copy(wsa[:, FOUT:], ps_wa[:])

        # per x-tile: load (2 DMA queues), transpose, [h|s12] matmul, srow matmul
        for t in range(T):
            xn = xin.tile([P, FIN], F32, tag="xn")
            dma_eng = nc.sync if t % 2 == 0 else nc.gpsimd
            dma_eng.dma_start(xn[:], x[t * P:(t + 1) * P, :])
            psT = ps_tr.tile([P, P], F32, tag="ps_t")
            nc.tensor.transpose(psT[:], xn[:], ident[:])
            # f32->f32r rounding copy on ACT (idle during setup)
            nc.scalar.copy(xT[:, t, :], psT[:])

        for t in range(T):
            ps_h = ps_mm.tile([P, FOUT + 2], F32, tag="ps_h")
            nc.tensor.matmul(ps_h[:], lhsT=xT[:, t, :], rhs=wsa[:],
                             start=True, stop=True)
            nc.vector.tensor_copy(hs12[:, t, :], ps_h[:])

            # score rows for this tile: (2, 128) = wa.T @ x_t.T
            ps_r = ps_mm.tile([2, P], F32, tag="ps_r")
            nc.tensor.matmul(ps_r[:], lhsT=wsa[:, FOUT:], rhs=xT[:, t, :],
                             start=True, stop=True)
            nc.vector.tensor_copy(srow[:, t * P:(t + 1) * P], ps_r[:])

        # broadcast s1 row to all 128 partitions via K=1 matmul
        for c in range(NC):
            sl = slice(c * 512, (c + 1) * 512)
            ps_b = ps_mm.tile([P, 512], F32, tag="ps_h")
            nc.tensor.matmul(ps_b[:], lhsT=ones_row[:], rhs=srow[0:1, sl],
                             start=True, stop=True)
            nc.vector.tensor_copy(s1b[:, sl], ps_b[:])

    # setup PSUM pools released; output accumulators take the banks
    ps_out = ctx.enter_context(tc.tile_pool(name="ps_out", bufs=1, space="PSUM"))
    hp_ps = [ps_out.tile([FOUT, 512], F32, tag=f"hp{c}", name=f"hp{c}")
             for c in range(NC)]

    # ---- main: per j-tile lrelu -> exp(+denom) -> scale h -> accumulate out ----
    for t in range(T):
        s2c = hs12[:, t, FOUT + 1:FOUT + 2]
        if t in ACT_TILES:
            l_t = lpool.tile([P, N], F32, tag="l")
            nc.scalar.activation(l_t[:], s1b[:], AF.Prelu,
                                 bias=s2c, scale=1.0, alpha=NEG_SLOPE)
        else:
            z02 = lpool.tile([P, N], F32, tag="z02")
            nc.gpsimd.tensor_scalar(z02[:], s1b[:], s2c, NEG_SLOPE,
                                    op0=ALU.add, op1=ALU.mult)
            l_t = lpool.tile([P, N], F32, tag="l")
            nc.vector.scalar_tensor_tensor(l_t[:], in0=s1b[:], scalar=s2c,
                                           in1=z02[:], op0=ALU.add, op1=ALU.max)

        den_t = dpool.tile([P, 1], F32, tag="den")
        nc.scalar.activation(p_all[:, t, :], l_t[:], AF.Exp, accum_out=den_t[:])

        rden_t = dpool.tile([P, 1], F32, tag="rden")
        nc.vector.reciprocal(rden_t[:], den_t[:])
        nc.vector.tensor_scalar_mul(hs_bf[:, t, :], hs12[:, t, 0:FOUT], rden_t[:])

        for c in range(NC):
            nc.tensor.matmul(hp_ps[c][:], lhsT=hs_bf[:, t, :],
                             rhs=p_all[:, t, c * 512:(c + 1) * 512],
                             start=(t == 0), stop=(t == T - 1))

    # ---- epilogue: leakyrelu on ACT straight from PSUM, DMA out transposed ----
    for c in range(NC):
        sl = slice(c * 512, (c + 1) * 512)
        nc.scalar.activation(o_sb[:, sl], hp_ps[c][:], AF.Prelu,
                             bias=0.0, scale=1.0, alpha=NEG_SLOPE)
        nc.sync.dma_start(out[:, sl], o_sb[:, sl])


_NC_CACHE = {}


def _build_nc():
    if "nc" in _NC_CACHE:
        return _NC_CACHE["nc"]
    nc = bacc.Bacc("TRN2", target_bir_lowering=False, debug=False)
    x = nc.dram_tensor("x", (N, FIN), F32, kind="ExternalInput").ap()
    w = nc.dram_tensor("w", (FIN, FOUT), F32, kind="ExternalInput").ap()
    a = nc.dram_tensor("a", (2 * FOUT, 1), F32, kind="ExternalInput").ap()
    # transposed output; the host un-transposes
    out = nc.dram_tensor("out", (FOUT, N), F32, kind="ExternalOutput").ap()
    with tile.TileContext(nc) as tc:
        _gat_body(tc, x, w, a, out)
    nc.compile()
    _NC_CACHE["nc"] = nc
    return nc


def kernel(x, W, a):
    x = np.ascontiguousarray(np.asarray(x), dtype=np.float32)
    W = np.ascontiguousarray(np.asarray(W), dtype=np.float32)
    a = np.ascontiguousarray(np.asarray(a), dtype=np.float32)
    assert x.shape == (N_CORES, N, FIN), x.shape
    nc = _build_nc()
    in_maps = [{"x": x[c], "w": W, "a": a} for c in range(N_CORES)]
    res = run_bass_kernel_spmd(nc, in_maps, core_ids=list(range(N_CORES)))
    return np.stack([res.results[c]["out"].T.copy() for c in range(N_CORES)], axis=0)
